# revision 29
# baseline (speedup 1.0000x reference)
"""Trainium2 Bass kernel for single-head attention (N=16384, F=512, M=128),
sequence-parallel over 8 NeuronCores.

Strategy (hardcoded, self-contained):
- Each core owns 2048 query rows. K/V projections are computed redundantly on
  every core (fp8 DoubleRow makes them cheap) -> no collectives.
- Host passes x^T in fp8 per core, rotated so the core's own query columns are
  always columns 0:2048 -> identical SPMD graph on all cores. Softmax sums are
  permutation-invariant over keys, so rotated K/V order is harmless.
- Projection weights are pre-scaled by 16 on the host so fp8e4m3 stays in its
  normal range; the 1/256 compensation folds into the exp() scale and Wo/16.
- bk drops out of softmax exactly; bv passes through the attention average
  unchanged, so the host folds it into bo' = bv @ Wo + bo.
- Scores are computed transposed (S^T = K @ Q^T, layout [j, q]) so the exp
  output E^T feeds V^T E directly with no transposes. E and V are fp8; the
  O-accumulation matmuls use DoubleRow (two key-tiles contracted per matmul).
- Softmax denominators: E tiles are accumulated elementwise on the Vector and
  GpSimd engines (split to keep both under the PE's pace), then reduced
  across partitions by tiny fp32 matmuls directly into per-partition [q,1]
  layout; 1/sum is applied after the (linear) output projection.
"""

import math
import sys

import numpy as np

for _p in ("/opt/trn_rl_repo", "/opt/pypackages"):
    if _p not in sys.path:
        sys.path.append(_p)

import ml_dtypes

N = 16384
F = 512
MD = 128
P = 128
NCORES = 8
NQ = N // NCORES      # 2048 query rows per core
QB = 512              # q-block (one PSUM bank of fp32)
NQB = NQ // QB        # 4
JT = 128              # j (key) tile
NJT = N // JT         # 128
FK = F // P           # 4 contraction tiles over features
CH = 512              # xt streaming chunk (j columns)
NCH = N // CH         # 32
GK = 16               # j-tiles per SBUF super-group
NG = NJT // GK        # 8
WS = 16.0             # host-side fp8 weight pre-scale
SCALE = 1.0 / math.sqrt(MD) / (WS * WS)

_BF16 = ml_dtypes.bfloat16
_FP8 = ml_dtypes.float8_e4m3fn


def _build():
    import concourse.bass as bass  # noqa: F401
    import concourse.tile as tile
    from concourse import bacc, mybir

    f32 = mybir.dt.float32
    bf16 = mybir.dt.bfloat16
    fp8 = mybir.dt.float8e4
    DR = mybir.MatmulPerfMode.DoubleRow
    AF = mybir.ActivationFunctionType
    ALU = mybir.AluOpType

    nc = bacc.Bacc("TRN2", target_bir_lowering=False, debug=False,
                   num_devices=NCORES)

    xt = nc.declare_dram_parameter("xt", [F, N], fp8, isOutput=False)
    wq = nc.declare_dram_parameter("wq", [F, MD], fp8, isOutput=False)
    wk = nc.declare_dram_parameter("wk", [F, MD], fp8, isOutput=False)
    wv = nc.declare_dram_parameter("wv", [F, F], fp8, isOutput=False)
    wo = nc.declare_dram_parameter("wo", [F, F], bf16, isOutput=False)
    bq = nc.declare_dram_parameter("bq", [MD, 1], f32, isOutput=False)
    bo = nc.declare_dram_parameter("bo", [1, F], f32, isOutput=False)
    out = nc.declare_dram_parameter("out", [NQ, F], f32, isOutput=True)

    with tile.TileContext(nc) as tc:
        with (
            tc.tile_pool(name="persist", bufs=1) as pp,
            tc.tile_pool(name="stream", bufs=4) as sp,
            tc.tile_pool(name="work", bufs=3) as wkp,
            tc.tile_pool(name="pssc", bufs=3, space="PSUM") as ps_sc,
            tc.tile_pool(name="pso", bufs=4, space="PSUM") as ps_o,
            tc.tile_pool(name="psesum", bufs=1, space="PSUM") as ps_es,
        ):
            # ---- persistent constants (vector/scalar DMA queues so the
            # gpsimd xt stream is not serialized behind them) --------------
            wq_a = pp.tile([P, FK, MD], fp8, tag="wqa")
            wk_a = pp.tile([P, FK, MD], fp8, tag="wka")
            wv_a = pp.tile([P, FK, F], fp8, tag="wva")
            wo_t = [pp.tile([P, F], bf16, tag=f"wo{k}", name=f"wo{k}")
                    for k in range(FK)]
            for k in range(FK):
                nc.sync.dma_start(out=wk_a[:, k, :], in_=wk[k * P:(k + 1) * P, :])
                nc.sync.dma_start(out=wv_a[:, k, :], in_=wv[k * P:(k + 1) * P, :])
            for k in range(FK):
                nc.scalar.dma_start(out=wq_a[:, k, :], in_=wq[k * P:(k + 1) * P, :])
                nc.scalar.dma_start(out=wo_t[k][:], in_=wo[k * P:(k + 1) * P, :])
            bq_t = pp.tile([MD, 1], f32, tag="bq")
            nc.scalar.dma_start(out=bq_t[:], in_=bq[:])
            bo_r = pp.tile([P, F], f32, tag="bor")
            nc.scalar.dma_start(out=bo_r[:], in_=bo[:].to_broadcast((P, F)))
            ones_f = pp.tile([P, 1], bf16, tag="ones")
            nc.vector.memset(ones_f[:], 1.0)
            id2 = pp.tile([P, 2, P], fp8, tag="id2")
            from concourse.masks import make_identity
            make_identity(nc, id2[:, 0, :])
            make_identity(nc, id2[:, 1, :])

            # ---- persistent activations -----------------------------------
            ktg = [pp.tile([P, GK * JT], bf16, tag=f"ktg{g}", name=f"ktg{g}")
                   for g in range(NG)]
            vg = [pp.tile([P, GK * F], fp8, tag=f"vg{g}", name=f"vg{g}")
                  for g in range(NG)]
            qt = pp.tile([P, NQ], bf16, tag="qt")

            # ---- PE warmup during the initial DMA wait (HAM un-throttle) --
            warm_ps = ps_sc.tile([P, P], f32, tag="sc", name="warm_ps")
            for wi in range(20):
                nc.tensor.matmul(warm_ps[:], id2[:, 0, :], id2[:, 0, :],
                                 start=(wi == 0), stop=(wi == 19))
            warm_s = pp.tile([P, P], bf16, tag="warms")
            nc.scalar.copy(warm_s[:], warm_ps[:])

            # ---- prologue: project Q^T, K^T, V (fp8 DoubleRow) ------------
            for ch in range(NCH):
                xtc = sp.tile([P, FK, CH], fp8, tag="xtc")
                xt4 = xt.rearrange("(k p) n -> p k n", p=P)
                dma_eng = nc.gpsimd if ch % 2 == 0 else nc.sync
                if ch < 2:
                    for k in range(FK):
                        dma_eng.dma_start(
                            out=xtc[:, k, :],
                            in_=xt[k * P:(k + 1) * P, ch * CH:(ch + 1) * CH])
                else:
                    dma_eng.dma_start(
                        out=xtc[:], in_=xt4[:, :, ch * CH:(ch + 1) * CH])
                g, off = ch // 4, (ch % 4) * CH
                pk = ps_es.tile([P, CH], f32, tag="esum", name="pk")
                for h in range(2):
                    nc.tensor.matmul(pk[:], wk_a[:, 2 * h:2 * h + 2, :],
                                     xtc[:, 2 * h:2 * h + 2, :],
                                     start=(h == 0), stop=(h == 1), perf_mode=DR)
                nc.scalar.copy(ktg[g][:, off:off + CH], pk[:])
                if ch < NQ // CH:
                    pq = ps_sc.tile([P, CH], f32, tag="sc", name="pq")
                    for h in range(2):
                        nc.tensor.matmul(pq[:], wq_a[:, 2 * h:2 * h + 2, :],
                                         xtc[:, 2 * h:2 * h + 2, :],
                                         start=(h == 0), stop=(h == 1),
                                         perf_mode=DR)
                    nc.scalar.activation(qt[:, ch * CH:(ch + 1) * CH], pq[:],
                                         AF.Identity, bias=bq_t[:], scale=1.0)
                for js in range(CH // JT):
                    jt_g = ch * (CH // JT) + js
                    voff = (jt_g % GK) * F
                    pv = ps_o.tile([P, F], f32, tag="oacc", name="pv")
                    for h in range(2):
                        nc.tensor.matmul(
                            pv[:], xtc[:, 2 * h:2 * h + 2, js * JT:(js + 1) * JT],
                            wv_a[:, 2 * h:2 * h + 2, :],
                            start=(h == 0), stop=(h == 1), perf_mode=DR)
                    if jt_g % 2 == 0:
                        nc.vector.tensor_copy(vg[jt_g // GK][:, voff:voff + F],
                                              pv[:])
                    else:
                        nc.scalar.copy(vg[jt_g // GK][:, voff:voff + F], pv[:])

            # ---- attention: flat pipeline over all (q-block, key-pair) ----
            # Sums of E are split: DVE handles 32 pairs/qb, GpSimd 16, and the
            # PE 16 (DoubleRow ones-matmul into a PSUM accumulator). Score
            # matmuls and epilogues pipeline across q-block boundaries.
            NP2 = NJT // 2
            # p_i%8 -> engine: 1 pair PE (esum), 5 DVE, 2 GpSimd
            SUMS_PAT = {1: "D", 2: "D", 3: "G", 4: "D", 5: "D", 6: "D",
                        7: "G"}

            def scores(gjt):
                qbb, jt_i = gjt // NJT, gjt % NJT
                g, r = jt_i // GK, jt_i % GK
                psc = ps_sc.tile([P, QB], f32, tag="sc", name="psc")
                nc.tensor.matmul(psc[:], ktg[g][:, r * JT:(r + 1) * JT],
                                 qt[:, qbb * QB:(qbb + 1) * QB],
                                 start=True, stop=True)
                return psc

            pending = {j: scores(j) for j in range(3)}
            state = {}
            deferred = [None]

            def epilogue(st):
                ot = st["ot"]
                esb = wkp.tile([P, QB], bf16, tag="esb", bufs=2, name="esb")
                nc.vector.tensor_copy(esb[:], st["esum"][:])
                recip_p = wkp.tile([P, QB // P], f32, tag="recipp", bufs=2,
                                   name="recip_p")
                acc_d, acc_g = st["acc_d"], st["acc_g"]
                for qs in range(QB // P):
                    pt = ps_sc.tile([P, 1], f32, tag="sc", name="pt")
                    srcs = [acc_d[:, qs * P:(qs + 1) * P],
                            acc_d[:, QB + qs * P:QB + (qs + 1) * P],
                            acc_g[:, qs * P:(qs + 1) * P],
                            acc_g[:, QB + qs * P:QB + (qs + 1) * P],
                            esb[:, qs * P:(qs + 1) * P]]
                    for si, s in enumerate(srcs):
                        nc.tensor.matmul(pt[:], s, ones_f[:],
                                         start=(si == 0), stop=(si == 4))
                    nc.vector.reciprocal(recip_p[:, qs:qs + 1], pt[:])
                    pf = ps_sc.tile([P, F], f32, tag="sc", name="pf")
                    for ft in range(FK):
                        nc.tensor.matmul(
                            pf[:], ot[:, ft * QB + qs * P:ft * QB + (qs + 1) * P],
                            wo_t[ft][:], start=(ft == 0), stop=(ft == FK - 1))
                    out_t = wkp.tile([P, F], f32, tag="outt", bufs=2, name="out_t")
                    nc.vector.scalar_tensor_tensor(
                        out_t[:], pf[:], recip_p[:, qs:qs + 1], bo_r[:],
                        ALU.mult, ALU.add)
                    row0 = st["qb"] * QB + qs * P
                    nc.sync.dma_start(out=out[row0:row0 + P, :], in_=out_t[:])

            for gp_i in range(NQB * NP2):
                qb, p_i = gp_i // NP2, gp_i % NP2
                if p_i == 0:
                    state = {
                        "qb": qb,
                        "po": [ps_o.tile([P, QB], f32, tag="oacc", name="oacc")
                               for _ in range(FK)],
                        "esum": ps_es.tile([P, QB], f32, tag="esum",
                                           name="esum"),
                        "acc_d": wkp.tile([P, 2 * QB], bf16, tag="accd", bufs=2,
                                          name="acc_d"),
                        "acc_g": wkp.tile([P, 2 * QB], bf16, tag="accg", bufs=2,
                                          name="acc_g"),
                        "seen": {"d": False, "g": False},
                    }
                jt0 = 2 * p_i
                g, r0 = jt0 // GK, jt0 % GK
                etp = wkp.tile([P, 2 * QB], fp8, tag="et", bufs=6)
                for h in range(2):
                    psc = pending.pop(qb * NJT + jt0 + h)
                    nc.scalar.activation(etp[:, h * QB:(h + 1) * QB], psc[:],
                                         AF.Exp, scale=SCALE)
                    nxt = qb * NJT + jt0 + h + 3
                    if nxt < NQB * NJT:
                        pending[nxt] = scores(nxt)
                et3 = etp.rearrange("p (h q) -> p h q", h=2)
                if p_i % 8 == 0:
                    nc.tensor.matmul(state["esum"][:], id2[:], et3,
                                     start=(p_i == 0), stop=(p_i == NP2 - 8),
                                     perf_mode=DR)
                else:
                    kind = SUMS_PAT[p_i % 8]
                    eng, acc, key = ((nc.vector, state["acc_d"], "d")
                                     if kind == "D"
                                     else (nc.gpsimd, state["acc_g"], "g"))
                    if not state["seen"][key]:
                        eng.tensor_copy(acc[:], etp[:])
                        state["seen"][key] = True
                    else:
                        eng.tensor_tensor(acc[:], acc[:], etp[:], ALU.add)
                vg4 = vg[g].rearrange("p (t h f) -> p t h f", h=2, f=F)
                for ft in range(FK):
                    nc.tensor.matmul(
                        po_t := state["po"][ft][:],
                        vg4[:, r0 // 2, :, ft * P:(ft + 1) * P],
                        et3, start=(p_i == 0), stop=(p_i == NP2 - 1),
                        perf_mode=DR)
                if p_i == 1 and deferred[0] is not None:
                    epilogue(deferred[0])
                    deferred[0] = None
                if p_i == NP2 - 1:
                    ot = wkp.tile([P, FK * QB], bf16, tag="ot", bufs=2, name="ot")
                    for ft in range(FK):
                        if ft % 2 == 0:
                            nc.scalar.copy(ot[:, ft * QB:(ft + 1) * QB],
                                           state["po"][ft][:])
                        else:
                            nc.vector.tensor_copy(ot[:, ft * QB:(ft + 1) * QB],
                                                  state["po"][ft][:])
                    state["ot"] = ot
                    deferred[0] = state
            epilogue(deferred[0])

    nc.compile()
    return nc


_CACHED = {}


def _get_nc():
    if "nc" not in _CACHED:
        _CACHED["nc"] = _build()
    return _CACHED["nc"]


def _make_in_maps(x, Wq, bq, Wk, bk, Wv, bv, Wo, bo):
    x = np.asarray(x, dtype=np.float32)
    xt_full = np.ascontiguousarray(x.T)                     # [F, N] f32
    wq_8 = (WS * np.asarray(Wq, np.float32)).astype(_FP8)
    wk_8 = (WS * np.asarray(Wk, np.float32)).astype(_FP8)
    wv_8 = (WS * np.asarray(Wv, np.float32)).astype(_FP8)
    wo_b = (np.asarray(Wo, np.float32) / WS).astype(_BF16)
    bq_h = (WS * np.asarray(bq, np.float32)).reshape(MD, 1).astype(np.float32)
    bo_p = (np.asarray(bv, np.float64) @ np.asarray(Wo, np.float64)
            + np.asarray(bo, np.float64)).astype(np.float32).reshape(1, F)

    in_maps = []
    for c in range(NCORES):
        s = c * NQ
        xt_rot = np.concatenate([xt_full[:, s:], xt_full[:, :s]], axis=1)
        in_maps.append({
            "xt": np.ascontiguousarray(xt_rot).astype(_FP8),
            "wq": wq_8, "wk": wk_8, "wv": wv_8, "wo": wo_b,
            "bq": bq_h, "bo": bo_p,
        })
    return in_maps


def kernel(x, Wq, bq, Wk, bk, Wv, bv, Wo, bo):
    from concourse.bass_utils import run_bass_kernel_spmd

    in_maps = _make_in_maps(x, Wq, bq, Wk, bk, Wv, bv, Wo, bo)
    nc = _get_nc()
    res = run_bass_kernel_spmd(nc, in_maps, core_ids=list(range(NCORES)))
    return np.concatenate(
        [res.results[c]["out"] for c in range(NCORES)], axis=0)


def run_traced(x, Wq, bq, Wk, bk, Wv, bv, Wo, bo):
    """Like kernel() but with NTFF tracing; returns (output, exec_time_ns)."""
    from concourse.bass_utils import run_bass_kernel_spmd

    try:
        import ntff_shim
        ntff_shim.install()
    except ImportError:
        pass
    in_maps = _make_in_maps(x, Wq, bq, Wk, bk, Wv, bv, Wo, bo)
    nc = _get_nc()
    res = run_bass_kernel_spmd(nc, in_maps, core_ids=list(range(NCORES)),
                               trace=True)
    out = np.concatenate([res.results[c]["out"] for c in range(NCORES)], axis=0)
    return out, res.exec_time_ns


# revision 31
# speedup vs baseline: 1.0021x; 1.0021x over previous
"""Trainium2 Bass kernel for single-head attention (N=16384, F=512, M=128),
sequence-parallel over 8 NeuronCores.

Strategy (hardcoded, self-contained):
- Each core owns 2048 query rows. K/V projections are computed redundantly on
  every core (fp8 DoubleRow makes them cheap) -> no collectives.
- Host passes x^T in fp8 per core, rotated so the core's own query columns are
  always columns 0:2048 -> identical SPMD graph on all cores. Softmax sums are
  permutation-invariant over keys, so rotated K/V order is harmless.
- Projection weights are pre-scaled by 16 on the host so fp8e4m3 stays in its
  normal range; the 1/256 compensation folds into the exp() scale and Wo/16.
- bk drops out of softmax exactly; bv passes through the attention average
  unchanged, so the host folds it into bo' = bv @ Wo + bo.
- Scores are computed transposed (S^T = K @ Q^T, layout [j, q]) so the exp
  output E^T feeds V^T E directly with no transposes. E and V are fp8; the
  O-accumulation matmuls use DoubleRow (two key-tiles contracted per matmul).
- Softmax denominators: E tiles are accumulated elementwise on the Vector and
  GpSimd engines (split to keep both under the PE's pace), then reduced
  across partitions by tiny fp32 matmuls directly into per-partition [q,1]
  layout; 1/sum is applied after the (linear) output projection.
"""

import math
import sys

import numpy as np

for _p in ("/opt/trn_rl_repo", "/opt/pypackages"):
    if _p not in sys.path:
        sys.path.append(_p)

import ml_dtypes

N = 16384
F = 512
MD = 128
P = 128
NCORES = 8
NQ = N // NCORES      # 2048 query rows per core
QB = 512              # q-block (one PSUM bank of fp32)
NQB = NQ // QB        # 4
JT = 128              # j (key) tile
NJT = N // JT         # 128
FK = F // P           # 4 contraction tiles over features
CH = 512              # xt streaming chunk (j columns)
NCH = N // CH         # 32
GK = 16               # j-tiles per SBUF super-group
NG = NJT // GK        # 8
WS = 16.0             # host-side fp8 weight pre-scale
SCALE = 1.0 / math.sqrt(MD) / (WS * WS)

_BF16 = ml_dtypes.bfloat16
_FP8 = ml_dtypes.float8_e4m3fn


def _build():
    import concourse.bass as bass  # noqa: F401
    import concourse.tile as tile
    from concourse import bacc, mybir

    f32 = mybir.dt.float32
    bf16 = mybir.dt.bfloat16
    fp8 = mybir.dt.float8e4
    DR = mybir.MatmulPerfMode.DoubleRow
    AF = mybir.ActivationFunctionType
    ALU = mybir.AluOpType

    nc = bacc.Bacc("TRN2", target_bir_lowering=False, debug=False,
                   num_devices=NCORES)

    xt = nc.declare_dram_parameter("xt", [F, N], fp8, isOutput=False)
    wq = nc.declare_dram_parameter("wq", [F, MD], fp8, isOutput=False)
    wk = nc.declare_dram_parameter("wk", [F, MD], fp8, isOutput=False)
    wv = nc.declare_dram_parameter("wv", [F, F], fp8, isOutput=False)
    wo = nc.declare_dram_parameter("wo", [F, F], bf16, isOutput=False)
    bq = nc.declare_dram_parameter("bq", [MD, 1], f32, isOutput=False)
    bo = nc.declare_dram_parameter("bo", [1, F], f32, isOutput=False)
    out = nc.declare_dram_parameter("out", [NQ, F], f32, isOutput=True)

    with tile.TileContext(nc) as tc:
        with (
            tc.tile_pool(name="persist", bufs=1) as pp,
            tc.tile_pool(name="stream", bufs=4) as sp,
            tc.tile_pool(name="work", bufs=3) as wkp,
            tc.tile_pool(name="pssc", bufs=3, space="PSUM") as ps_sc,
            tc.tile_pool(name="pso", bufs=4, space="PSUM") as ps_o,
            tc.tile_pool(name="psesum", bufs=1, space="PSUM") as ps_es,
        ):
            # ---- persistent constants (vector/scalar DMA queues so the
            # gpsimd xt stream is not serialized behind them) --------------
            wq_a = pp.tile([P, FK, MD], fp8, tag="wqa")
            wk_a = pp.tile([P, FK, MD], fp8, tag="wka")
            wv_a = pp.tile([P, FK, F], fp8, tag="wva")
            wo_t = [pp.tile([P, F], bf16, tag=f"wo{k}", name=f"wo{k}")
                    for k in range(FK)]
            for k in range(FK):
                nc.sync.dma_start(out=wk_a[:, k, :], in_=wk[k * P:(k + 1) * P, :])
                nc.sync.dma_start(out=wv_a[:, k, :], in_=wv[k * P:(k + 1) * P, :])
            for k in range(FK):
                nc.scalar.dma_start(out=wq_a[:, k, :], in_=wq[k * P:(k + 1) * P, :])
                nc.scalar.dma_start(out=wo_t[k][:], in_=wo[k * P:(k + 1) * P, :])
            bq_t = pp.tile([MD, 1], f32, tag="bq")
            nc.scalar.dma_start(out=bq_t[:], in_=bq[:])
            bo_r = pp.tile([P, F], f32, tag="bor")
            nc.scalar.dma_start(out=bo_r[:], in_=bo[:].to_broadcast((P, F)))
            ones_f = pp.tile([P, 1], bf16, tag="ones")
            nc.vector.memset(ones_f[:], 1.0)
            id2 = pp.tile([P, 2, P], fp8, tag="id2")
            from concourse.masks import make_identity
            make_identity(nc, id2[:, 0, :])
            make_identity(nc, id2[:, 1, :])

            # ---- persistent activations -----------------------------------
            ktg = [pp.tile([P, GK * JT], bf16, tag=f"ktg{g}", name=f"ktg{g}")
                   for g in range(NG)]
            vg = [pp.tile([P, GK * F], fp8, tag=f"vg{g}", name=f"vg{g}")
                  for g in range(NG)]
            qt = pp.tile([P, NQ], bf16, tag="qt")

            # ---- PE warmup during the initial DMA wait (HAM un-throttle) --
            warm_ps = ps_sc.tile([P, P], f32, tag="sc", name="warm_ps")
            for wi in range(20):
                nc.tensor.matmul(warm_ps[:], id2[:, 0, :], id2[:, 0, :],
                                 start=(wi == 0), stop=(wi == 19))
            warm_s = pp.tile([P, P], bf16, tag="warms")
            nc.scalar.copy(warm_s[:], warm_ps[:])

            # ---- prologue: project Q^T, K^T, V (fp8 DoubleRow) ------------
            for ch in range(NCH):
                xtc = sp.tile([P, FK, CH], fp8, tag="xtc")
                xt4 = xt.rearrange("(k p) n -> p k n", p=P)
                dma_eng = nc.gpsimd if ch % 2 == 0 else nc.sync
                if ch < 2:
                    for k in range(FK):
                        dma_eng.dma_start(
                            out=xtc[:, k, :],
                            in_=xt[k * P:(k + 1) * P, ch * CH:(ch + 1) * CH])
                else:
                    dma_eng.dma_start(
                        out=xtc[:], in_=xt4[:, :, ch * CH:(ch + 1) * CH])
                g, off = ch // 4, (ch % 4) * CH
                pk = ps_es.tile([P, CH], f32, tag="esum", name="pk")
                for h in range(2):
                    nc.tensor.matmul(pk[:], wk_a[:, 2 * h:2 * h + 2, :],
                                     xtc[:, 2 * h:2 * h + 2, :],
                                     start=(h == 0), stop=(h == 1), perf_mode=DR)
                nc.scalar.copy(ktg[g][:, off:off + CH], pk[:])
                if ch < NQ // CH:
                    pq = ps_sc.tile([P, CH], f32, tag="sc", name="pq")
                    for h in range(2):
                        nc.tensor.matmul(pq[:], wq_a[:, 2 * h:2 * h + 2, :],
                                         xtc[:, 2 * h:2 * h + 2, :],
                                         start=(h == 0), stop=(h == 1),
                                         perf_mode=DR)
                    nc.scalar.activation(qt[:, ch * CH:(ch + 1) * CH], pq[:],
                                         AF.Identity, bias=bq_t[:], scale=1.0)
                for js in range(CH // JT):
                    jt_g = ch * (CH // JT) + js
                    voff = (jt_g % GK) * F
                    pv = ps_o.tile([P, F], f32, tag="oacc", name="pv")
                    for h in range(2):
                        nc.tensor.matmul(
                            pv[:], xtc[:, 2 * h:2 * h + 2, js * JT:(js + 1) * JT],
                            wv_a[:, 2 * h:2 * h + 2, :],
                            start=(h == 0), stop=(h == 1), perf_mode=DR)
                    if jt_g % 2 == 0:
                        nc.vector.tensor_copy(vg[jt_g // GK][:, voff:voff + F],
                                              pv[:])
                    else:
                        nc.scalar.copy(vg[jt_g // GK][:, voff:voff + F], pv[:])

            # ---- attention: flat pipeline over all (q-block, key-pair) ----
            # Sums of E are split: DVE handles 32 pairs/qb, GpSimd 16, and the
            # PE 16 (DoubleRow ones-matmul into a PSUM accumulator). Score
            # matmuls and epilogues pipeline across q-block boundaries.
            NP2 = NJT // 2
            # p_i%8 -> engine: 1 pair PE (esum), 5 DVE, 2 GpSimd
            SUMS_PAT = {1: "D", 2: "D", 3: "G", 4: "D", 5: "D", 6: "D",
                        7: "G"}

            def scores(gjt):
                qbb, jt_i = gjt // NJT, gjt % NJT
                g, r = jt_i // GK, jt_i % GK
                psc = ps_sc.tile([P, QB], f32, tag="sc", name="psc")
                nc.tensor.matmul(psc[:], ktg[g][:, r * JT:(r + 1) * JT],
                                 qt[:, qbb * QB:(qbb + 1) * QB],
                                 start=True, stop=True)
                return psc

            pending = {j: scores(j) for j in range(3)}
            state = {}
            deferred = [None]

            def epilogue(st):
                ot = st["ot"]
                esb = wkp.tile([P, QB], bf16, tag="esb", bufs=2, name="esb")
                nc.vector.tensor_copy(esb[:], st["esum"][:])
                recip_p = wkp.tile([P, QB // P], f32, tag="recipp", bufs=2,
                                   name="recip_p")
                acc_d, acc_g = st["acc_d"], st["acc_g"]
                for qs in range(QB // P):
                    pt = ps_sc.tile([P, 1], f32, tag="sc", name="pt")
                    srcs = [acc_d[:, qs * P:(qs + 1) * P],
                            acc_d[:, QB + qs * P:QB + (qs + 1) * P],
                            acc_g[:, qs * P:(qs + 1) * P],
                            acc_g[:, QB + qs * P:QB + (qs + 1) * P],
                            esb[:, qs * P:(qs + 1) * P]]
                    for si, s in enumerate(srcs):
                        nc.tensor.matmul(pt[:], s, ones_f[:],
                                         start=(si == 0), stop=(si == 4))
                    nc.vector.reciprocal(recip_p[:, qs:qs + 1], pt[:])
                    pf = ps_sc.tile([P, F], f32, tag="sc", name="pf")
                    for ft in range(FK):
                        nc.tensor.matmul(
                            pf[:], ot[:, ft * QB + qs * P:ft * QB + (qs + 1) * P],
                            wo_t[ft][:], start=(ft == 0), stop=(ft == FK - 1))
                    out_t = wkp.tile([P, F], f32, tag="outt", bufs=2, name="out_t")
                    nc.vector.scalar_tensor_tensor(
                        out_t[:], pf[:], recip_p[:, qs:qs + 1], bo_r[:],
                        ALU.mult, ALU.add)
                    row0 = st["qb"] * QB + qs * P
                    nc.sync.dma_start(out=out[row0:row0 + P, :], in_=out_t[:])

            for gp_i in range(NQB * NP2):
                qb, p_i = gp_i // NP2, gp_i % NP2
                if p_i == 0:
                    state = {
                        "qb": qb,
                        "po": [ps_o.tile([P, QB], f32, tag="oacc", name="oacc")
                               for _ in range(FK)],
                        "esum": ps_es.tile([P, QB], f32, tag="esum",
                                           name="esum"),
                        "acc_d": wkp.tile([P, 2 * QB], bf16, tag="accd", bufs=2,
                                          name="acc_d"),
                        "acc_g": wkp.tile([P, 2 * QB], bf16, tag="accg", bufs=2,
                                          name="acc_g"),
                        "seen": {"d": False, "g": False},
                    }
                jt0 = 2 * p_i
                g, r0 = jt0 // GK, jt0 % GK
                etp = wkp.tile([P, 2 * QB], fp8, tag="et", bufs=6)
                for h in range(2):
                    psc = pending.pop(qb * NJT + jt0 + h)
                    nc.scalar.activation(etp[:, h * QB:(h + 1) * QB], psc[:],
                                         AF.Exp, scale=SCALE)
                    nxt = qb * NJT + jt0 + h + 3
                    if nxt < NQB * NJT:
                        pending[nxt] = scores(nxt)
                et3 = etp.rearrange("p (h q) -> p h q", h=2)
                if p_i % 8 == 0:
                    nc.tensor.matmul(state["esum"][:], id2[:], et3,
                                     start=(p_i == 0), stop=(p_i == NP2 - 8),
                                     perf_mode=DR)
                else:
                    kind = SUMS_PAT[p_i % 8]
                    eng, acc, key = ((nc.vector, state["acc_d"], "d")
                                     if kind == "D"
                                     else (nc.gpsimd, state["acc_g"], "g"))
                    if not state["seen"][key]:
                        eng.tensor_copy(acc[:], etp[:])
                        state["seen"][key] = True
                    else:
                        eng.tensor_tensor(acc[:], acc[:], etp[:], ALU.add)
                vg4 = vg[g].rearrange("p (t h f) -> p t h f", h=2, f=F)
                for ft in range(FK):
                    nc.tensor.matmul(
                        po_t := state["po"][ft][:],
                        vg4[:, r0 // 2, :, ft * P:(ft + 1) * P],
                        et3, start=(p_i == 0), stop=(p_i == NP2 - 1),
                        perf_mode=DR)
                if p_i == 1 and deferred[0] is not None:
                    epilogue(deferred[0])
                    deferred[0] = None
                if p_i == NP2 - 1:
                    ot = wkp.tile([P, FK * QB], bf16, tag="ot", bufs=2, name="ot")
                    for ft in range(FK):
                        if ft % 2 == 0:
                            nc.scalar.copy(ot[:, ft * QB:(ft + 1) * QB],
                                           state["po"][ft][:])
                        else:
                            nc.vector.tensor_copy(ot[:, ft * QB:(ft + 1) * QB],
                                                  state["po"][ft][:])
                    state["ot"] = ot
                    deferred[0] = state
            epilogue(deferred[0])

    nc.compile()
    return nc


_CACHED = {}


def _get_nc():
    if "nc" not in _CACHED:
        _CACHED["nc"] = _build()
    return _CACHED["nc"]


def _make_in_maps(x, Wq, bq, Wk, bk, Wv, bv, Wo, bo):
    x = np.asarray(x, dtype=np.float32)
    xt_full = np.ascontiguousarray(x.T)                     # [F, N] f32
    wq_8 = (WS * np.asarray(Wq, np.float32)).astype(_FP8)
    wk_8 = (WS * np.asarray(Wk, np.float32)).astype(_FP8)
    wv_8 = (WS * np.asarray(Wv, np.float32)).astype(_FP8)
    wo_b = (np.asarray(Wo, np.float32) / WS).astype(_BF16)
    bq_h = (WS * np.asarray(bq, np.float32)).reshape(MD, 1).astype(np.float32)
    bo_p = (np.asarray(bv, np.float64) @ np.asarray(Wo, np.float64)
            + np.asarray(bo, np.float64)).astype(np.float32).reshape(1, F)

    in_maps = []
    for c in range(NCORES):
        s = c * NQ
        xt_rot = np.concatenate([xt_full[:, s:], xt_full[:, :s]], axis=1)
        in_maps.append({
            "xt": np.ascontiguousarray(xt_rot).astype(_FP8),
            "wq": wq_8, "wk": wk_8, "wv": wv_8, "wo": wo_b,
            "bq": bq_h, "bo": bo_p,
        })
    return in_maps


def kernel(x, Wq, bq, Wk, bk, Wv, bv, Wo, bo):
    from concourse.bass_utils import run_bass_kernel_spmd

    in_maps = _make_in_maps(x, Wq, bq, Wk, bk, Wv, bv, Wo, bo)
    nc = _get_nc()
    res = run_bass_kernel_spmd(nc, in_maps, core_ids=list(range(NCORES)))
    return np.concatenate(
        [res.results[c]["out"] for c in range(NCORES)], axis=0)


def run_traced(x, Wq, bq, Wk, bk, Wv, bv, Wo, bo):
    """Like kernel() but with NTFF tracing; returns (output, exec_time_ns)."""
    from concourse.bass_utils import run_bass_kernel_spmd

    try:
        import ntff_shim
        ntff_shim.install()
    except ImportError:
        pass
    in_maps = _make_in_maps(x, Wq, bq, Wk, bk, Wv, bv, Wo, bo)
    nc = _get_nc()
    res = run_bass_kernel_spmd(nc, in_maps, core_ids=list(range(NCORES)),
                               trace=True)
    out = np.concatenate([res.results[c]["out"] for c in range(NCORES)], axis=0)
    return out, res.exec_time_ns
